# revision 46
# baseline (speedup 1.0000x reference)
"""Trainium2 Bass kernel for EnhancedMultHeadAttention (8-core SPMD).

Reference semantics (bugs preserved):
    q_p = q @ wq_w.T + wq_b
    v_p = v @ wv_w.T + wv_b
    k_p = v_p @ wv_w.T + wv_b          # k input unused
    logits = (q_p . k_p^T per head) / 8
    attention_weights = softmax(logits)
    attn_out = logits @ v_p_heads      # raw logits, NOT softmax weights
    output = concat(attn_out) @ dense_w.T + dense_b
    returns (output, attention_weights)

Sharding: core c -> batch b = c//2, head-half hh = c%2 (8 heads, 512 cols).

Two associativity collapses (both exact up to fp reordering):
  * attn_out_h = (q_p_h/8) @ (k_p_h^T @ v_p_h)  - the Gram matrix k^T v is
    only 64x64 per head because no softmax sits in between.
  * k_p[:, cols] = v @ W2^T + b2 with W2 = wv[cols,:] @ wv (host-computed),
    b2 = wv[cols,:] @ wv_b + wv_b[cols] - so k_pT streams directly off the
    vT input and the core never materialises the full v_p (only its own
    512 columns, needed as attention values).

All matmuls run as float32r (full-rate fp32 on the PE at N=512).
Projections are computed transposed ([d, s]); the natural-layout copies
needed by the Gram stage come from cheap PE transposes of the
already-biased k_pT / v_pT. The wv-column dimension is permuted on the
host ("core's 512 dims first") so per-core slices are fixed slices and
the SPMD program is identical across cores.
"""

import numpy as np

import concourse.bass as bass
import concourse.mybir as mybir
import concourse.tile as tile
from concourse.tile_rust import add_dep_helper
from concourse import bacc
from concourse._compat import axon_active
from concourse.bass_utils import run_bass_kernel_spmd

B, S, D, H, DH = 4, 1024, 1024, 16, 64
N_CORES = 8
HH = H // 2          # heads per core
CD = D // 2          # concat cols per core
F32 = mybir.dt.float32
F32R = mybir.dt.float32r
AF = mybir.ActivationFunctionType

TRACE = False        # test.py flips this to profile
LIMIT = 99           # emission milestone cutoff (for timeline bisection)

_cache = {}


def _build():
    """Build + schedule the SPMD Bass program (identical on all cores)."""
    nc = bacc.Bacc("TRN2", target_bir_lowering=False, debug=False,
                   num_devices=N_CORES)

    # ---- DRAM I/O (per-core data, same shapes everywhere) ----
    # vT rows are permuted on the host: dim order perm = [core cols, rest].
    vT_d = nc.dram_tensor("vT", [D, S], F32R, kind="ExternalInput")
    qT_d = nc.dram_tensor("qT", [D, S], F32R, kind="ExternalInput")
    w2T_d = nc.dram_tensor("w2T", [D, CD], F32R, kind="ExternalInput")
    wvTh_d = nc.dram_tensor("wvTh", [D, CD], F32R, kind="ExternalInput")
    wqTh_d = nc.dram_tensor("wqTh", [4, D, 128], F32R, kind="ExternalInput")
    dwTh_d = nc.dram_tensor("dwTh", [CD, D], F32R, kind="ExternalInput")
    bvh_d = nc.dram_tensor("bvh", [128, 4], F32, kind="ExternalInput")
    bq_d = nc.dram_tensor("bq", [128, 4], F32, kind="ExternalInput")
    bk2_d = nc.dram_tensor("bk2", [128, 4], F32, kind="ExternalInput")
    ident_d = nc.dram_tensor("ident", [128, 128], F32R, kind="ExternalInput")

    aw_d = nc.dram_tensor("aw", [HH, S, S], F32, kind="ExternalOutput")
    part_d = nc.dram_tensor("part", [S, D], F32, kind="ExternalOutput")

    with tile.TileContext(nc) as tc:
        with (
            tc.tile_pool(name="sb", bufs=1) as sb,
            tc.tile_pool(name="wk", bufs=1) as wk,
            tc.tile_pool(name="ps", bufs=4, space="PSUM") as ps,
            tc.tile_pool(name="psl", bufs=2, space="PSUM") as psl,
        ):
            # ---- persistent SBUF tensors (tags control slot reuse) ----
            w2T_sb = sb.tile([128, 8, CD], F32R, tag="w2T")     # 16KB
            vT_sb = sb.tile([128, 8, S], F32R, tag="slotB")     # 32KB slot
            qT_sb = sb.tile([128, 8, S], F32R, tag="slotC")     # 32KB slot
            wvTh_sb = sb.tile([128, 8, CD], F32R, tag="slotA")  # 16KB slot
            wqTh_sb = sb.tile([128, 8, CD], F32R, tag="slotD")  # 16KB slot
            v_pT_sb = sb.tile([128, 4, S], F32R, tag="vpT")     # 16KB
            q_pT_sb = sb.tile([128, 4, S], F32R, tag="qpT")     # 16KB
            k_pT_sb = sb.tile([128, 4, S], F32R, tag="kpT")     # 16KB
            m1_sb = sb.tile([128, 4, 128], F32R, tag="m1")      # 2KB
            bvh_sb = sb.tile([128, 4], F32, tag="bvh")
            bq_sb = sb.tile([128, 4], F32, tag="bq")
            bk2_sb = sb.tile([128, 4], F32, tag="bk2")
            ident_sb = sb.tile([128, 128], F32R, tag="ident")
            dwTh_sb = sb.tile([128, 4, D], F32R, tag="w2T")  # w2T's slot

            def load2(sb_tile, dram, kk2, eng=None):
                """One DMA covering two 128-row k-slices of [1024, w]."""
                src = dram[kk2 * 256:(kk2 + 1) * 256, :].rearrange(
                    "(b p) n -> p b n", p=128)
                (eng or nc.sync).dma_start(
                    sb_tile[:, 2 * kk2:2 * kk2 + 2, :], src)

            def load_wq_block(j):
                src = wqTh_d[j].rearrange("(b p) n -> p b n", p=128)
                nc.sync.dma_start(wqTh_sb[:, :, j * 128:(j + 1) * 128], src)

            # ---- load phase, ordered by when compute needs the bytes ----
            # 1) w2T+vT gate the k_pT stream (and everything after);
            # 2) wq block0 + qT gate the q stream / first logits;
            # 3) the rest trails behind the already-running softmax stream.
            for kk2 in range(4):
                load2(w2T_sb, w2T_d, kk2)
                load2(vT_sb, vT_d, kk2)
            nc.sync.dma_start(bk2_sb[:], bk2_d[:])
            nc.sync.dma_start(bq_sb[:], bq_d[:])
            nc.sync.dma_start(bvh_sb[:], bvh_d[:])
            nc.sync.dma_start(ident_sb[:], ident_d[:])
            load_wq_block(0)
            for kk2 in range(4):
                load2(qT_sb, qT_d, kk2)
            for j in range(1, 4):
                load_wq_block(j)
            for kk2 in range(4):
                load2(wvTh_sb, wvTh_d, kk2)
            for kk2 in range(2):
                load2(dwTh_sb, dwTh_d, kk2)

            # ---- k_pT[dout in cols, s] = w2T.T @ vT + b2 (kk-streamed) ----
            # 8 concurrent PSUM groups chase the arriving w2T/vT slices:
            # 4 x [128,512] ps_main tiles + 2 x [128,1024] psl tiles used
            # as two bank-disjoint groups each.
            if LIMIT >= 1:
                accs = []
                for _ in range(4):
                    accs.append(ps.tile([128, 512], F32, tag="ps_main",
                                        name="kacc"))
                for _ in range(2):
                    big = psl.tile([128, S], F32, tag="ps_log", name="kaccl")
                    accs += [big[:, 0:512], big[:, 512:1024]]
                for kk in range(8):
                    g = 0
                    for j in range(4):
                        for sh in range(2):
                            nc.tensor.matmul(
                                accs[g],
                                w2T_sb[:, kk, j * 128:(j + 1) * 128],
                                vT_sb[:, kk, sh * 512:(sh + 1) * 512],
                                start=(kk == 0), stop=(kk == 7),
                            )
                            g += 1
                g = 0
                for j in range(4):
                    for sh in range(2):
                        nc.vector.tensor_scalar_add(
                            k_pT_sb[:, j, sh * 512:(sh + 1) * 512], accs[g],
                            bk2_sb[:, j:j + 1],
                        )
                        g += 1

            # ---- q_pT head-pair 0, kk-streamed off the arriving qT ----
            stream0_last = [None]

            def emit_qpT_stream0():
                qbig = psl.tile([128, S], F32, tag="ps_log", name="qacc")
                acc0 = qbig[:, 0:512]
                acc1 = qbig[:, 512:1024]
                for kk in range(8):
                    for sh, acc in ((0, acc0), (1, acc1)):
                        mm = nc.tensor.matmul(
                            acc,
                            wqTh_sb[:, kk, 0:128],
                            qT_sb[:, kk, sh * 512:(sh + 1) * 512],
                            start=(kk == 0), stop=(kk == 7),
                        )
                        stream0_last[0] = mm.ins
                for sh, acc in ((0, acc0), (1, acc1)):
                    nc.scalar.activation(
                        q_pT_sb[:, 0, sh * 512:(sh + 1) * 512], acc,
                        AF.Identity, bias=bq_sb[:, 0:1],
                    )

            def emit_qpT(j, shs=(0, 1)):
                for sh in shs:
                    acc = ps.tile([128, 512], F32, tag="ps_main")
                    for kk in range(8):
                        mm = nc.tensor.matmul(
                            acc[:],
                            wqTh_sb[:, kk, j * 128:(j + 1) * 128],
                            qT_sb[:, kk, sh * 512:(sh + 1) * 512],
                            start=(kk == 0), stop=(kk == 7),
                        )
                        if kk == 0 and stream0_last[0] is not None:
                            add_dep_helper(mm.ins, stream0_last[0],
                                           sync=False,
                                           reason="after q stream")
                    nc.vector.tensor_scalar_add(
                        q_pT_sb[:, j, sh * 512:(sh + 1) * 512], acc[:],
                        bq_sb[:, j:j + 1],
                    )

            # ---- v_pT[dout in cols, s] = wvTh.T @ vT + bias ----
            def emit_vpT(j, shs=(0, 1)):
                for sh in shs:
                    acc = ps.tile([128, 512], F32, tag="ps_main")
                    for kk in range(8):
                        nc.tensor.matmul(
                            acc[:],
                            wvTh_sb[:, kk, j * 128:(j + 1) * 128],
                            vT_sb[:, kk, sh * 512:(sh + 1) * 512],
                            start=(kk == 0), stop=(kk == 7),
                        )
                    nc.vector.tensor_scalar_add(
                        v_pT_sb[:, j, sh * 512:(sh + 1) * 512], acc[:],
                        bvh_sb[:, j:j + 1],
                    )

            # Ordering gate: slack-phase matmul groups ordered after the
            # first logits matmul so the greedy scheduler cannot backfill
            # them ahead of the ACT-bound softmax stream.
            gate = [None]
            gate_map = {}

            def order_after_logits(mm):
                if gate[0] is not None:
                    add_dep_helper(mm.ins, gate[0], sync=False,
                                   reason="keep softmax stream ahead")
                return mm

            # ---- logits + softmax; one s-tile per call -> one aw DMA ----
            def emit_logits(hp, si):
                for par in range(2):            # even/odd head of the pair
                    h = 2 * hp + par
                    prange = slice(par * 64, par * 64 + 64)
                    tp = (par * 64, 0)
                    awt = wk.tile([128, S], F32, tag="awt", bufs=8)
                    accl = psl.tile([128, S], F32, tag="ps_log")
                    for th in range(2):
                        mm = nc.tensor.matmul(
                            accl[:, th * 512:(th + 1) * 512],
                            q_pT_sb[prange, hp, si * 128:(si + 1) * 128],
                            k_pT_sb[prange, hp, th * 512:(th + 1) * 512],
                            start=True, stop=True, tile_position=tp,
                        )
                        if gate[0] is None:
                            gate[0] = mm.ins
                        gate_map[(hp, si)] = mm.ins
                    sums = wk.tile([128, 2], F32, tag="sums", bufs=4)
                    nc.scalar.activation(
                        awt[:], accl[:], AF.Exp, accum_out=sums[:, 0:1],
                    )
                    nc.vector.reciprocal(sums[:, 1:2], sums[:, 0:1])
                    nc.vector.tensor_scalar_mul(awt[:], awt[:], sums[:, 1:2])
                    nc.sync.dma_start(
                        aw_d[h, si * 128:(si + 1) * 128, :], awt[:])

            # ---- natural-layout copies via PE transposes (bias included,
            # since v_pT/k_pT are already biased) ----
            v_pn_sb = sb.tile([128, 8, CD], F32R, tag="slotD")  # wqTh slot
            k_pn_sb = sb.tile([128, 8, CD], F32R, tag="slotC")  # qT's slot

            def emit_vpn2(hp):
                for ti in range(8):
                    acc = ps.tile([128, 128], F32R, tag="ps_main", name="tacc")
                    mm = nc.tensor.transpose(
                        acc[:], v_pT_sb[:, hp, ti * 128:(ti + 1) * 128],
                        ident_sb[:])
                    if ti == 0:
                        order_after_logits(mm)
                    nc.vector.tensor_copy(
                        v_pn_sb[:, ti, hp * 128:(hp + 1) * 128], acc[:])

            def emit_kpn2(hp):
                for ti in range(8):
                    acc = ps.tile([128, 128], F32R, tag="ps_main", name="tacc")
                    mm = nc.tensor.transpose(
                        acc[:], k_pT_sb[:, hp, ti * 128:(ti + 1) * 128],
                        ident_sb[:])
                    if ti == 0:
                        order_after_logits(mm)
                    nc.vector.tensor_copy(
                        k_pn_sb[:, ti, hp * 128:(hp + 1) * 128], acc[:])

            # ---- M1 pair Gram matrices: k_pair.T @ v_pair (diag blocks) ----
            def emit_m1(hp):
                accm = ps.tile([128, 128], F32, tag="ps_main")
                for ti in range(8):
                    nc.tensor.matmul(
                        accm[:],
                        k_pn_sb[:, ti, hp * 128:(hp + 1) * 128],
                        v_pn_sb[:, ti, hp * 128:(hp + 1) * 128],
                        start=(ti == 0), stop=(ti == 7),
                    )
                nc.vector.tensor_copy(m1_sb[:, hp, :], accm[:])

            # ---- attn_finalT[j, s]: lhsT=M1_h, rhs=q_hT (K=64, M=64) ----
            afT_sb = sb.tile([128, 4, S], F32R, tag="slotA")  # wvTh's slot

            def emit_af(hp):
                for sh in range(2):
                    for par in range(2):
                        h2 = slice(par * 64, par * 64 + 64)
                        acca = ps.tile([64, 512], F32, tag="ps_main")
                        nc.tensor.matmul(
                            acca[:],
                            m1_sb[h2, hp, par * 64:par * 64 + 64],
                            q_pT_sb[h2, hp, sh * 512:(sh + 1) * 512],
                            start=True, stop=True,
                            tile_position=(par * 64, 0),
                        )
                        nc.vector.tensor_copy(
                            afT_sb[par * 64:par * 64 + 64, hp,
                                   sh * 512:(sh + 1) * 512],
                            acca[:],
                        )

            # ---- dense partial: part[s, :] = afT.T @ dwTh ----
            def emit_dense(si):
                out_t = wk.tile([128, D], F32, tag="outt", bufs=2)
                for oh in range(2):
                    accd = ps.tile([128, 512], F32, tag="ps_main")
                    for hp in range(4):
                        nc.tensor.matmul(
                            accd[:],
                            afT_sb[:, hp, si * 128:(si + 1) * 128],
                            dwTh_sb[:, hp, oh * 512:(oh + 1) * 512],
                            start=(hp == 0), stop=(hp == 3),
                        )
                    nc.vector.tensor_copy(
                        out_t[:, oh * 512:(oh + 1) * 512], accd[:])
                nc.sync.dma_start(
                    part_d[si * 128:(si + 1) * 128, :], out_t[:])

            # ---- emission schedule ----
            if LIMIT >= 2:
                emit_qpT_stream0()
                for hp in range(4):
                    emit_logits(hp, 0)
                    if hp < 3:
                        emit_qpT(hp + 1, shs=(0,))
                    emit_logits(hp, 1)
                    if hp < 3:
                        emit_qpT(hp + 1, shs=(1,))
                    emit_vpT(hp)
            if LIMIT >= 3:
                for hp in range(4):
                    emit_logits(hp, 2)
                    emit_logits(hp, 3)
                    emit_vpn2(hp)
            if LIMIT >= 4:
                for hp in range(4):
                    emit_logits(hp, 4)
                    emit_logits(hp, 5)
                    emit_kpn2(hp)
            if LIMIT >= 5:
                for hp in range(4):
                    emit_m1(hp)
                    emit_af(hp)
            if LIMIT >= 6:
                for hp in range(4):
                    emit_logits(hp, 6)
                    emit_logits(hp, 7)
                    emit_dense(2 * hp)
                    emit_dense(2 * hp + 1)

    nc.compile()
    return nc


def _prep_inputs(v, k, q, wq_w, wq_b, wv_w, wv_b, dense_w, dense_b):
    del k
    f32 = np.float32
    wv = np.asarray(wv_w, dtype=f32)
    wvb = np.asarray(wv_b, dtype=f32)
    wqT_s = np.ascontiguousarray(np.asarray(wq_w).T, dtype=f32) / f32(8.0)
    dwT = np.ascontiguousarray(np.asarray(dense_w).T, dtype=f32)
    wq_b_s = np.asarray(wq_b).astype(f32) / f32(8.0)
    wvT = np.ascontiguousarray(wv.T)

    # weight variants depend only on the head-half, not the core
    halves = []
    ident = np.eye(128, dtype=f32)
    for hh in range(2):
        cols = np.arange(hh * CD, (hh + 1) * CD)
        perm = np.concatenate([cols, np.setdiff1d(np.arange(D), cols)])
        # k_p collapse: W2 = wv[cols,:] @ wv ; b2 = wv[cols,:] @ wv_b + wv_b
        W2 = wv[cols, :] @ wv                             # [CD, D]
        b2 = wv[cols, :] @ wvb + wvb[cols]
        halves.append({
            "perm": perm,
            "w2T": np.ascontiguousarray(W2.T[perm]),      # rows perm'd
            "wvTh": np.ascontiguousarray(wvT[np.ix_(perm, cols)]),
            "wqTh": np.ascontiguousarray(
                wqT_s[:, cols].reshape(D, 4, 128).transpose(1, 0, 2)),
            "dwTh": np.ascontiguousarray(dwT[cols, :]),
            "bvh": np.ascontiguousarray(wvb[cols].reshape(4, 128).T),
            "bq": np.ascontiguousarray(wq_b_s[cols].reshape(4, 128).T),
            "bk2": np.ascontiguousarray(b2.reshape(4, 128).T),
            "ident": ident,
        })

    in_maps = []
    for c in range(N_CORES):
        b, hh = c // 2, c % 2
        hv = halves[hh]
        m = {k2: v2 for k2, v2 in hv.items() if k2 != "perm"}
        m["vT"] = np.ascontiguousarray(np.asarray(v[b]).T[hv["perm"]],
                                       dtype=f32)
        m["qT"] = np.ascontiguousarray(np.asarray(q[b]).T, dtype=f32)
        in_maps.append(m)
    return in_maps


def _make_runner(nc):
    """Build the PJRT shard_map executable ONCE (run_bass_via_pjrt rebuilds
    it per call, paying re-trace + cache lookup each time)."""
    import jax
    from jax.experimental.shard_map import shard_map
    from jax.sharding import Mesh, NamedSharding, PartitionSpec

    from concourse.bass2jax import (
        _bass_exec_p, install_neuronx_cc_hook, partition_id_tensor)

    install_neuronx_cc_hook()
    part_name = nc.partition_id_tensor.name if nc.partition_id_tensor else None
    in_names, out_names, out_avals, zero_shapes = [], [], [], []
    for alloc in nc.m.functions[0].allocations:
        if not isinstance(alloc, mybir.MemoryLocationSet):
            continue
        name = alloc.memorylocations[0].name
        if alloc.kind == "ExternalInput":
            if name != part_name:
                in_names.append(name)
        elif alloc.kind == "ExternalOutput":
            out_names.append(name)
            shape = tuple(alloc.tensor_shape)
            dtype = mybir.dt.np(alloc.dtype)
            out_avals.append(jax.core.ShapedArray(shape, dtype))
            zero_shapes.append((shape, dtype))
    n_params = len(in_names)
    n_outs = len(out_names)
    all_names = in_names + out_names
    if part_name is not None:
        all_names = all_names + [part_name]

    def _body(*args):
        operands = list(args)
        if part_name is not None:
            operands.append(partition_id_tensor())
        return tuple(_bass_exec_p.bind(
            *operands,
            out_avals=tuple(out_avals),
            in_names=tuple(all_names),
            out_names=tuple(out_names),
            lowering_input_output_aliases=(),
            sim_require_finite=True,
            sim_require_nnan=True,
            nc=nc,
        ))

    devices = jax.devices()[:N_CORES]
    mesh = Mesh(np.asarray(devices), ("core",))
    donate = tuple(range(n_params, n_params + n_outs))
    fn = jax.jit(
        shard_map(_body, mesh=mesh,
                  in_specs=(PartitionSpec("core"),) * (n_params + n_outs),
                  out_specs=(PartitionSpec("core"),) * n_outs,
                  check_rep=False),
        donate_argnums=donate, keep_unused=True,
    )
    sharding = NamedSharding(mesh, PartitionSpec("core"))
    make_zeros = jax.jit(
        lambda: tuple(
            jax.numpy.zeros((N_CORES * s[0], *s[1:]), d)
            for s, d in zero_shapes),
        out_shardings=(sharding,) * n_outs,
    )

    def run(in_maps):
        concat_in = [
            np.concatenate([np.asarray(in_maps[c][n]) for c in range(N_CORES)],
                           axis=0)
            for n in in_names
        ]
        outs = fn(*concat_in, *make_zeros())
        return [
            {name: np.asarray(outs[i]).reshape(N_CORES, *out_avals[i].shape)[c]
             for i, name in enumerate(out_names)}
            for c in range(N_CORES)
        ]

    return run


def kernel(v, k, q, wq_w, wq_b, wv_w, wv_b, dense_w, dense_b):
    if "nc" not in _cache:
        _cache["nc"] = _build()
    nc = _cache["nc"]

    in_maps = _prep_inputs(v, k, q, wq_w, wq_b, wv_w, wv_b, dense_w, dense_b)
    if axon_active():
        if "runner" not in _cache:
            _cache["runner"] = _make_runner(nc)
        results = _cache["runner"](in_maps)
    else:
        res = run_bass_kernel_spmd(
            nc, in_maps, core_ids=list(range(N_CORES)), trace=TRACE,
        )
        _cache["last_result"] = res
        results = res.results

    out = np.empty((B, S, D), dtype=np.float32)
    aw = np.empty((B, H, S, S), dtype=np.float32)
    dense_b = np.asarray(dense_b, dtype=np.float32)
    for b in range(B):
        p0 = results[2 * b]["part"]
        p1 = results[2 * b + 1]["part"]
        out[b] = p0 + p1 + dense_b
        aw[b, :HH] = results[2 * b]["aw"]
        aw[b, HH:] = results[2 * b + 1]["aw"]
    return out, aw


# revision 50
# speedup vs baseline: 1.0100x; 1.0100x over previous
"""Trainium2 Bass kernel for EnhancedMultHeadAttention (8-core SPMD).

Reference semantics (bugs preserved):
    q_p = q @ wq_w.T + wq_b
    v_p = v @ wv_w.T + wv_b
    k_p = v_p @ wv_w.T + wv_b          # k input unused
    logits = (q_p . k_p^T per head) / 8
    attention_weights = softmax(logits)
    attn_out = logits @ v_p_heads      # raw logits, NOT softmax weights
    output = concat(attn_out) @ dense_w.T + dense_b
    returns (output, attention_weights)

Sharding: core c -> batch b = c//2, head-half hh = c%2 (8 heads, 512 cols).

Two associativity collapses (both exact up to fp reordering):
  * attn_out_h = (q_p_h/8) @ (k_p_h^T @ v_p_h)  - the Gram matrix k^T v is
    only 64x64 per head because no softmax sits in between.
  * k_p[:, cols] = v @ W2^T + b2 with W2 = wv[cols,:] @ wv (host-computed),
    b2 = wv[cols,:] @ wv_b + wv_b[cols] - so k_pT streams directly off the
    vT input and the core never materialises the full v_p (only its own
    512 columns, needed as attention values).

All matmuls run as float32r (full-rate fp32 on the PE at N=512).
Projections are computed transposed ([d, s]); the natural-layout copies
needed by the Gram stage come from cheap PE transposes of the
already-biased k_pT / v_pT. The wv-column dimension is permuted on the
host ("core's 512 dims first") so per-core slices are fixed slices and
the SPMD program is identical across cores.
"""

import numpy as np

import concourse.bass as bass
import concourse.mybir as mybir
import concourse.tile as tile
from concourse.tile_rust import add_dep_helper
from concourse import bacc
from concourse._compat import axon_active
from concourse.bass_utils import run_bass_kernel_spmd

B, S, D, H, DH = 4, 1024, 1024, 16, 64
N_CORES = 8
HH = H // 2          # heads per core
CD = D // 2          # concat cols per core
F32 = mybir.dt.float32
F32R = mybir.dt.float32r
AF = mybir.ActivationFunctionType

TRACE = False        # test.py flips this to profile
LIMIT = 99           # emission milestone cutoff (for timeline bisection)

_cache = {}


def _build():
    """Build + schedule the SPMD Bass program (identical on all cores)."""
    nc = bacc.Bacc("TRN2", target_bir_lowering=False, debug=False,
                   num_devices=N_CORES)

    # ---- DRAM I/O (per-core data, same shapes everywhere) ----
    # vT rows are permuted on the host: dim order perm = [core cols, rest].
    vT_d = nc.dram_tensor("vT", [D, S], F32R, kind="ExternalInput")
    qT_d = nc.dram_tensor("qT", [D, S], F32R, kind="ExternalInput")
    w2T_d = nc.dram_tensor("w2T", [D, CD], F32R, kind="ExternalInput")
    wvTh_d = nc.dram_tensor("wvTh", [D, CD], F32R, kind="ExternalInput")
    wqTh_d = nc.dram_tensor("wqTh", [4, D, 128], F32R, kind="ExternalInput")
    dwTh_d = nc.dram_tensor("dwTh", [CD, D], F32R, kind="ExternalInput")
    bvh_d = nc.dram_tensor("bvh", [128, 4], F32, kind="ExternalInput")
    bq_d = nc.dram_tensor("bq", [128, 4], F32, kind="ExternalInput")
    bk2_d = nc.dram_tensor("bk2", [128, 4], F32, kind="ExternalInput")
    ident_d = nc.dram_tensor("ident", [128, 128], F32R, kind="ExternalInput")

    aw_d = nc.dram_tensor("aw", [HH, S, S], F32, kind="ExternalOutput")
    part_d = nc.dram_tensor("part", [S, D], F32, kind="ExternalOutput")

    with tile.TileContext(nc) as tc:
        with (
            tc.tile_pool(name="sb", bufs=1) as sb,
            tc.tile_pool(name="wk", bufs=1) as wk,
            tc.tile_pool(name="ps", bufs=4, space="PSUM") as ps,
            tc.tile_pool(name="psl", bufs=2, space="PSUM") as psl,
        ):
            # ---- persistent SBUF tensors (tags control slot reuse) ----
            w2T_sb = sb.tile([128, 8, CD], F32R, tag="w2T")     # 16KB
            vT_sb = sb.tile([128, 8, S], F32R, tag="slotB")     # 32KB slot
            qT_sb = sb.tile([128, 8, S], F32R, tag="slotC")     # 32KB slot
            wvTh_sb = sb.tile([128, 8, CD], F32R, tag="slotA")  # 16KB slot
            wqTh_sb = sb.tile([128, 8, CD], F32R, tag="slotD")  # 16KB slot
            v_pT_sb = sb.tile([128, 4, S], F32R, tag="vpT")     # 16KB
            q_pT_sb = sb.tile([128, 4, S], F32R, tag="qpT")     # 16KB
            k_pT_sb = sb.tile([128, 4, S], F32R, tag="kpT")     # 16KB
            m1_sb = sb.tile([128, 4, 128], F32R, tag="m1")      # 2KB
            bvh_sb = sb.tile([128, 4], F32, tag="bvh")
            bq_sb = sb.tile([128, 4], F32, tag="bq")
            bk2_sb = sb.tile([128, 4], F32, tag="bk2")
            ident_sb = sb.tile([128, 128], F32R, tag="ident")
            dwTh_sb = sb.tile([128, 4, D], F32R, tag="w2T")  # w2T's slot

            def load2(sb_tile, dram, kk2, eng=None):
                """One DMA covering two 128-row k-slices of [1024, w]."""
                src = dram[kk2 * 256:(kk2 + 1) * 256, :].rearrange(
                    "(b p) n -> p b n", p=128)
                (eng or nc.sync).dma_start(
                    sb_tile[:, 2 * kk2:2 * kk2 + 2, :], src)

            def load_wq_block(j):
                src = wqTh_d[j].rearrange("(b p) n -> p b n", p=128)
                nc.sync.dma_start(wqTh_sb[:, :, j * 128:(j + 1) * 128], src)

            def load1(sb_tile, dram, kk):
                nc.sync.dma_start(sb_tile[:, kk, :],
                                  dram[kk * 128:(kk + 1) * 128, :])

            # ---- load phase, ordered by when compute needs the bytes ----
            # 1) w2T+vT gate the k_pT stream (and everything after); their
            #    first slices go as singles so the PE starts ~2us earlier;
            # 2) wq block0 + qT gate the q stream / first logits;
            # 3) the rest trails behind the already-running softmax stream.
            for kk in range(2):
                load1(w2T_sb, w2T_d, kk)
                load1(vT_sb, vT_d, kk)
            for kk in range(2, 8):
                load1(w2T_sb, w2T_d, kk)
                load1(vT_sb, vT_d, kk)
            nc.sync.dma_start(bk2_sb[:], bk2_d[:])
            load_wq_block(0)
            nc.sync.dma_start(bq_sb[:], bq_d[:])
            for kk2 in range(4):
                load2(qT_sb, qT_d, kk2)
            nc.sync.dma_start(bvh_sb[:], bvh_d[:])
            nc.sync.dma_start(ident_sb[:], ident_d[:])
            for j in range(1, 4):
                load_wq_block(j)
            for kk2 in range(4):
                load2(wvTh_sb, wvTh_d, kk2)
            for kk2 in range(2):
                load2(dwTh_sb, dwTh_d, kk2)

            # ---- k_pT[dout in cols, s] = w2T.T @ vT + b2 (kk-streamed) ----
            # 8 concurrent PSUM groups chase the arriving w2T/vT slices:
            # 4 x [128,512] ps_main tiles + 2 x [128,1024] psl tiles used
            # as two bank-disjoint groups each.
            if LIMIT >= 1:
                accs = []
                for _ in range(4):
                    accs.append(ps.tile([128, 512], F32, tag="ps_main",
                                        name="kacc"))
                for _ in range(2):
                    big = psl.tile([128, S], F32, tag="ps_log", name="kaccl")
                    accs += [big[:, 0:512], big[:, 512:1024]]
                for kk in range(8):
                    g = 0
                    for j in range(4):
                        for sh in range(2):
                            nc.tensor.matmul(
                                accs[g],
                                w2T_sb[:, kk, j * 128:(j + 1) * 128],
                                vT_sb[:, kk, sh * 512:(sh + 1) * 512],
                                start=(kk == 0), stop=(kk == 7),
                            )
                            g += 1
                g = 0
                for j in range(4):
                    for sh in range(2):
                        nc.vector.tensor_scalar_add(
                            k_pT_sb[:, j, sh * 512:(sh + 1) * 512], accs[g],
                            bk2_sb[:, j:j + 1],
                        )
                        g += 1

            # ---- q_pT head-pair 0, kk-streamed off the arriving qT ----
            stream0_last = [None]

            def emit_qpT_stream0():
                qbig = psl.tile([128, S], F32, tag="ps_log", name="qacc")
                acc0 = qbig[:, 0:512]
                acc1 = qbig[:, 512:1024]
                for kk in range(8):
                    for sh, acc in ((0, acc0), (1, acc1)):
                        mm = nc.tensor.matmul(
                            acc,
                            wqTh_sb[:, kk, 0:128],
                            qT_sb[:, kk, sh * 512:(sh + 1) * 512],
                            start=(kk == 0), stop=(kk == 7),
                        )
                        stream0_last[0] = mm.ins
                for sh, acc in ((0, acc0), (1, acc1)):
                    nc.scalar.activation(
                        q_pT_sb[:, 0, sh * 512:(sh + 1) * 512], acc,
                        AF.Identity, bias=bq_sb[:, 0:1],
                    )

            def emit_qpT(j, shs=(0, 1)):
                for sh in shs:
                    acc = ps.tile([128, 512], F32, tag="ps_main")
                    for kk in range(8):
                        mm = nc.tensor.matmul(
                            acc[:],
                            wqTh_sb[:, kk, j * 128:(j + 1) * 128],
                            qT_sb[:, kk, sh * 512:(sh + 1) * 512],
                            start=(kk == 0), stop=(kk == 7),
                        )
                        if kk == 0 and stream0_last[0] is not None:
                            add_dep_helper(mm.ins, stream0_last[0],
                                           sync=False,
                                           reason="after q stream")
                    nc.vector.tensor_scalar_add(
                        q_pT_sb[:, j, sh * 512:(sh + 1) * 512], acc[:],
                        bq_sb[:, j:j + 1],
                    )

            # ---- v_pT[dout in cols, s] = wvTh.T @ vT + bias ----
            def emit_vpT(j, shs=(0, 1)):
                for sh in shs:
                    acc = ps.tile([128, 512], F32, tag="ps_main")
                    for kk in range(8):
                        nc.tensor.matmul(
                            acc[:],
                            wvTh_sb[:, kk, j * 128:(j + 1) * 128],
                            vT_sb[:, kk, sh * 512:(sh + 1) * 512],
                            start=(kk == 0), stop=(kk == 7),
                        )
                    nc.vector.tensor_scalar_add(
                        v_pT_sb[:, j, sh * 512:(sh + 1) * 512], acc[:],
                        bvh_sb[:, j:j + 1],
                    )

            # Ordering gate: slack-phase matmul groups ordered after the
            # first logits matmul so the greedy scheduler cannot backfill
            # them ahead of the ACT-bound softmax stream.
            gate = [None]
            gate_map = {}

            def order_after_logits(mm):
                if gate[0] is not None:
                    add_dep_helper(mm.ins, gate[0], sync=False,
                                   reason="keep softmax stream ahead")
                return mm

            # ---- logits + softmax; one s-tile per call -> one aw DMA ----
            def emit_logits(hp, si):
                for par in range(2):            # even/odd head of the pair
                    h = 2 * hp + par
                    prange = slice(par * 64, par * 64 + 64)
                    tp = (par * 64, 0)
                    awt = wk.tile([128, S], F32, tag="awt", bufs=8)
                    accl = psl.tile([128, S], F32, tag="ps_log")
                    for th in range(2):
                        mm = nc.tensor.matmul(
                            accl[:, th * 512:(th + 1) * 512],
                            q_pT_sb[prange, hp, si * 128:(si + 1) * 128],
                            k_pT_sb[prange, hp, th * 512:(th + 1) * 512],
                            start=True, stop=True, tile_position=tp,
                        )
                        if gate[0] is None:
                            gate[0] = mm.ins
                        gate_map[(hp, si)] = mm.ins
                    sums = wk.tile([128, 2], F32, tag="sums", bufs=8)
                    nc.scalar.activation(
                        awt[:], accl[:], AF.Exp, accum_out=sums[:, 0:1],
                    )
                    nc.vector.reciprocal(sums[:, 1:2], sums[:, 0:1])
                    nc.vector.tensor_scalar_mul(awt[:], awt[:], sums[:, 1:2])
                    nc.sync.dma_start(
                        aw_d[h, si * 128:(si + 1) * 128, :], awt[:])

            # ---- natural-layout copies via PE transposes (bias included,
            # since v_pT/k_pT are already biased) ----
            v_pn_sb = sb.tile([128, 8, CD], F32R, tag="slotD")  # wqTh slot
            k_pn_sb = sb.tile([128, 8, CD], F32R, tag="slotC")  # qT's slot

            def emit_vpn2(hp):
                for ti in range(8):
                    acc = ps.tile([128, 128], F32R, tag="ps_main", name="tacc")
                    mm = nc.tensor.transpose(
                        acc[:], v_pT_sb[:, hp, ti * 128:(ti + 1) * 128],
                        ident_sb[:])
                    if ti == 0:
                        order_after_logits(mm)
                    nc.vector.tensor_copy(
                        v_pn_sb[:, ti, hp * 128:(hp + 1) * 128], acc[:])

            def emit_kpn2(hp):
                for ti in range(8):
                    acc = ps.tile([128, 128], F32R, tag="ps_main", name="tacc")
                    mm = nc.tensor.transpose(
                        acc[:], k_pT_sb[:, hp, ti * 128:(ti + 1) * 128],
                        ident_sb[:])
                    if ti == 0:
                        order_after_logits(mm)
                    nc.vector.tensor_copy(
                        k_pn_sb[:, ti, hp * 128:(hp + 1) * 128], acc[:])

            # ---- M1 pair Gram matrices: k_pair.T @ v_pair (diag blocks) ----
            def emit_m1(hp):
                accm = ps.tile([128, 128], F32, tag="ps_main")
                for ti in range(8):
                    nc.tensor.matmul(
                        accm[:],
                        k_pn_sb[:, ti, hp * 128:(hp + 1) * 128],
                        v_pn_sb[:, ti, hp * 128:(hp + 1) * 128],
                        start=(ti == 0), stop=(ti == 7),
                    )
                nc.vector.tensor_copy(m1_sb[:, hp, :], accm[:])

            # ---- attn_finalT[j, s]: lhsT=M1_h, rhs=q_hT (K=64, M=64) ----
            afT_sb = sb.tile([128, 4, S], F32R, tag="slotA")  # wvTh's slot

            def emit_af(hp):
                for sh in range(2):
                    for par in range(2):
                        h2 = slice(par * 64, par * 64 + 64)
                        acca = ps.tile([64, 512], F32, tag="ps_main")
                        nc.tensor.matmul(
                            acca[:],
                            m1_sb[h2, hp, par * 64:par * 64 + 64],
                            q_pT_sb[h2, hp, sh * 512:(sh + 1) * 512],
                            start=True, stop=True,
                            tile_position=(par * 64, 0),
                        )
                        nc.vector.tensor_copy(
                            afT_sb[par * 64:par * 64 + 64, hp,
                                   sh * 512:(sh + 1) * 512],
                            acca[:],
                        )

            # ---- dense partial: part[s, :] = afT.T @ dwTh ----
            def emit_dense(si):
                out_t = wk.tile([128, D], F32, tag="outt", bufs=2)
                for oh in range(2):
                    accd = ps.tile([128, 512], F32, tag="ps_main")
                    for hp in range(4):
                        nc.tensor.matmul(
                            accd[:],
                            afT_sb[:, hp, si * 128:(si + 1) * 128],
                            dwTh_sb[:, hp, oh * 512:(oh + 1) * 512],
                            start=(hp == 0), stop=(hp == 3),
                        )
                    nc.vector.tensor_copy(
                        out_t[:, oh * 512:(oh + 1) * 512], accd[:])
                nc.sync.dma_start(
                    part_d[si * 128:(si + 1) * 128, :], out_t[:])

            # ---- emission schedule ----
            if LIMIT >= 2:
                emit_qpT_stream0()
                for hp in range(4):
                    emit_logits(hp, 0)
                    if hp < 3:
                        emit_qpT(hp + 1, shs=(0,))
                    emit_logits(hp, 1)
                    if hp < 3:
                        emit_qpT(hp + 1, shs=(1,))
                    emit_vpT(hp)
            if LIMIT >= 3:
                for hp in range(4):
                    emit_logits(hp, 2)
                    emit_logits(hp, 3)
                    emit_vpn2(hp)
            if LIMIT >= 4:
                for hp in range(4):
                    emit_logits(hp, 4)
                    emit_logits(hp, 5)
                    emit_kpn2(hp)
            if LIMIT >= 5:
                for hp in range(4):
                    emit_m1(hp)
                    emit_af(hp)
            if LIMIT >= 6:
                for hp in range(4):
                    emit_logits(hp, 6)
                    emit_logits(hp, 7)
                    emit_dense(2 * hp)
                    emit_dense(2 * hp + 1)

    nc.compile()
    return nc


def _prep_inputs(v, k, q, wq_w, wq_b, wv_w, wv_b, dense_w, dense_b):
    del k
    f32 = np.float32
    wv = np.asarray(wv_w, dtype=f32)
    wvb = np.asarray(wv_b, dtype=f32)
    wqT_s = np.ascontiguousarray(np.asarray(wq_w).T, dtype=f32) / f32(8.0)
    dwT = np.ascontiguousarray(np.asarray(dense_w).T, dtype=f32)
    wq_b_s = np.asarray(wq_b).astype(f32) / f32(8.0)
    wvT = np.ascontiguousarray(wv.T)

    # weight variants depend only on the head-half, not the core
    halves = []
    ident = np.eye(128, dtype=f32)
    for hh in range(2):
        cols = np.arange(hh * CD, (hh + 1) * CD)
        perm = np.concatenate([cols, np.setdiff1d(np.arange(D), cols)])
        # k_p collapse: W2 = wv[cols,:] @ wv ; b2 = wv[cols,:] @ wv_b + wv_b
        W2 = wv[cols, :] @ wv                             # [CD, D]
        b2 = wv[cols, :] @ wvb + wvb[cols]
        halves.append({
            "perm": perm,
            "w2T": np.ascontiguousarray(W2.T[perm]),      # rows perm'd
            "wvTh": np.ascontiguousarray(wvT[np.ix_(perm, cols)]),
            "wqTh": np.ascontiguousarray(
                wqT_s[:, cols].reshape(D, 4, 128).transpose(1, 0, 2)),
            "dwTh": np.ascontiguousarray(dwT[cols, :]),
            "bvh": np.ascontiguousarray(wvb[cols].reshape(4, 128).T),
            "bq": np.ascontiguousarray(wq_b_s[cols].reshape(4, 128).T),
            "bk2": np.ascontiguousarray(b2.reshape(4, 128).T),
            "ident": ident,
        })

    in_maps = []
    for c in range(N_CORES):
        b, hh = c // 2, c % 2
        hv = halves[hh]
        m = {k2: v2 for k2, v2 in hv.items() if k2 != "perm"}
        m["vT"] = np.ascontiguousarray(np.asarray(v[b]).T[hv["perm"]],
                                       dtype=f32)
        m["qT"] = np.ascontiguousarray(np.asarray(q[b]).T, dtype=f32)
        in_maps.append(m)
    return in_maps


def _make_runner(nc):
    """Build the PJRT shard_map executable ONCE (run_bass_via_pjrt rebuilds
    it per call, paying re-trace + cache lookup each time)."""
    import jax
    from jax.experimental.shard_map import shard_map
    from jax.sharding import Mesh, NamedSharding, PartitionSpec

    from concourse.bass2jax import (
        _bass_exec_p, install_neuronx_cc_hook, partition_id_tensor)

    install_neuronx_cc_hook()
    part_name = nc.partition_id_tensor.name if nc.partition_id_tensor else None
    in_names, out_names, out_avals, zero_shapes = [], [], [], []
    for alloc in nc.m.functions[0].allocations:
        if not isinstance(alloc, mybir.MemoryLocationSet):
            continue
        name = alloc.memorylocations[0].name
        if alloc.kind == "ExternalInput":
            if name != part_name:
                in_names.append(name)
        elif alloc.kind == "ExternalOutput":
            out_names.append(name)
            shape = tuple(alloc.tensor_shape)
            dtype = mybir.dt.np(alloc.dtype)
            out_avals.append(jax.core.ShapedArray(shape, dtype))
            zero_shapes.append((shape, dtype))
    n_params = len(in_names)
    n_outs = len(out_names)
    all_names = in_names + out_names
    if part_name is not None:
        all_names = all_names + [part_name]

    def _body(*args):
        operands = list(args)
        if part_name is not None:
            operands.append(partition_id_tensor())
        return tuple(_bass_exec_p.bind(
            *operands,
            out_avals=tuple(out_avals),
            in_names=tuple(all_names),
            out_names=tuple(out_names),
            lowering_input_output_aliases=(),
            sim_require_finite=True,
            sim_require_nnan=True,
            nc=nc,
        ))

    devices = jax.devices()[:N_CORES]
    mesh = Mesh(np.asarray(devices), ("core",))
    donate = tuple(range(n_params, n_params + n_outs))
    fn = jax.jit(
        shard_map(_body, mesh=mesh,
                  in_specs=(PartitionSpec("core"),) * (n_params + n_outs),
                  out_specs=(PartitionSpec("core"),) * n_outs,
                  check_rep=False),
        donate_argnums=donate, keep_unused=True,
    )
    sharding = NamedSharding(mesh, PartitionSpec("core"))
    make_zeros = jax.jit(
        lambda: tuple(
            jax.numpy.zeros((N_CORES * s[0], *s[1:]), d)
            for s, d in zero_shapes),
        out_shardings=(sharding,) * n_outs,
    )

    def run(in_maps):
        concat_in = [
            np.concatenate([np.asarray(in_maps[c][n]) for c in range(N_CORES)],
                           axis=0)
            for n in in_names
        ]
        outs = fn(*concat_in, *make_zeros())
        return [
            {name: np.asarray(outs[i]).reshape(N_CORES, *out_avals[i].shape)[c]
             for i, name in enumerate(out_names)}
            for c in range(N_CORES)
        ]

    return run


def kernel(v, k, q, wq_w, wq_b, wv_w, wv_b, dense_w, dense_b):
    if "nc" not in _cache:
        _cache["nc"] = _build()
    nc = _cache["nc"]

    in_maps = _prep_inputs(v, k, q, wq_w, wq_b, wv_w, wv_b, dense_w, dense_b)
    if axon_active():
        if "runner" not in _cache:
            _cache["runner"] = _make_runner(nc)
        results = _cache["runner"](in_maps)
    else:
        res = run_bass_kernel_spmd(
            nc, in_maps, core_ids=list(range(N_CORES)), trace=TRACE,
        )
        _cache["last_result"] = res
        results = res.results

    out = np.empty((B, S, D), dtype=np.float32)
    aw = np.empty((B, H, S, S), dtype=np.float32)
    dense_b = np.asarray(dense_b, dtype=np.float32)
    for b in range(B):
        p0 = results[2 * b]["part"]
        p1 = results[2 * b + 1]["part"]
        out[b] = p0 + p1 + dense_b
        aw[b, :HH] = results[2 * b]["aw"]
        aw[b, HH:] = results[2 * b + 1]["aw"]
    return out, aw
